# revision 10
# baseline (speedup 1.0000x reference)
"""ConvNet+Izhikevich spiking net on 8 TRN2 NeuronCores.

Data-parallel over batch: B=16 -> 8 cores x B_shard=2. Per core the
activation sequence [T=32, Bs=2, 256, 256] lives in SBUF as fp8e4 spike
frames (in-place across layers). The 3x3 conv runs on the TensorEngine as
fp8 DoubleRow matmuls (each fuses the two H-chunk contributions for one
W-shift at 0.5 cycles/row). The Izhikevich state update is split across
all four engines and software-pipelined across the 4 layers (wavefront over
(layer, t) with no barriers):

  psV = conv(z) * 0.25 + (-0.25*u)     (PE: 12 DR mms + 2 f32r diag mms)
  s   = 0.01*(v + 112.5)^2             (ACT Square, scale=0.1 bias=11.25)
  v   = (s + -91.5625) + psV           (Pool STT; v_pre, const folded)
  z   = v > 30  -> acts[t] fp8         (DVE TS)
  psU = 0.001*v_old + 6*z              (PE: f32r diag mm + fp8 diag mm)
  v   = z ? -65 : v                    (DVE copy_predicated)
  u   = 0.995*u + psU                  (Pool STT)

with RAW state u (u' = 0.995u + 0.001v + 6z has no constant term; the
v-recursion constant -91.5625 rides the v_pre STT scalar slot).
"""

import numpy as np

T, BS, H, W, P = 32, 2, 256, 256, 128
NCH = H // P          # 2 partition chunks of H
FR = W + 2            # padded frame row: 258
FREE = NCH * BS * FR  # 1032 free elems per act frame
NL = 4
KSC = 0.25            # TAU_INV * DT

_CACHE = {}


def _build():
    import concourse.bass as bass
    import concourse.bacc as bacc
    import concourse.mybir as mybir
    from concourse.tile import TileContext

    f32 = mybir.dt.float32
    f32r = mybir.dt.float32r
    bf16 = mybir.dt.bfloat16
    fp8 = mybir.dt.float8e4
    i8 = mybir.dt.int8
    Alu = mybir.AluOpType
    Act = mybir.ActivationFunctionType
    DR = mybir.MatmulPerfMode.DoubleRow

    nc = bacc.Bacc("TRN2", target_bir_lowering=False)
    x_d = nc.dram_tensor("x", [T, BS, H, W], f32, kind="ExternalInput")
    bands_d = nc.dram_tensor("bands", [NL, 2, 3, 2, P, P], fp8, kind="ExternalInput")
    diags_d = nc.dram_tensor("diags", [3, P, P], bf16, kind="ExternalInput")
    diag6r_d = nc.dram_tensor("diag6r", [P, P], f32, kind="ExternalInput")
    diag6_d = nc.dram_tensor("diag6", [P, P], fp8, kind="ExternalInput")
    out_d = nc.dram_tensor("out", [T, BS, H, W], f32, kind="ExternalOutput")

    x_r = x_d.ap().rearrange("t b (c p) w -> t b p c w", p=P)
    o_r = out_d.ap().rearrange("t b (c p) w -> t b p c w", p=P)
    bands_r = bands_d.ap().rearrange("l mc dw two k m -> k l mc dw two m")
    diags_r = diags_d.ap().rearrange("d k m -> k d m")
    diag6_r = diag6_d.ap().rearrange("k m -> k m")
    diag6r_r = diag6r_d.ap().rearrange("k m -> k m")

    with TileContext(nc) as tc:
        with (
            tc.tile_pool(name="acts", bufs=T) as act_pool,
            tc.tile_pool(name="state", bufs=1) as st_pool,
            tc.tile_pool(name="consts", bufs=1) as c_pool,
            tc.tile_pool(name="stg", bufs=1) as stg_pool,
            tc.tile_pool(name="psum", bufs=2, space="PSUM") as ps_pool,
        ):
            acts = [
                act_pool.tile([P, FREE], fp8, tag="acts", name=f"act{i}")
                for i in range(T)
            ]
            vs = [st_pool.tile([P, NCH * BS * W], bf16, tag=f"v{l}", name=f"v{l}") for l in range(NL)]
            us = [st_pool.tile([P, NCH * BS * W], bf16, tag=f"u{l}", name=f"u{l}") for l in range(NL)]
            sps = [st_pool.tile([P, NCH * BS * W], f32, tag=f"sp{l}", name=f"sp{l}") for l in range(NL)]
            ctile = c_pool.tile([P, NCH * BS * W], bf16, tag="ctile")
            sqb = c_pool.tile([P, 1], f32, tag="sqb")
            bands = c_pool.tile([P, NL * 2 * 3 * 2 * P], fp8, tag="bands")
            diags = c_pool.tile([P, 3 * P], bf16, tag="diags")
            d6r = c_pool.tile([P, P], f32r, tag="d6r")
            dstage = c_pool.tile([P, P], f32, tag="dstage")
            diag6 = c_pool.tile([P, P], fp8, tag="diag6")
            sts = [
                stg_pool.tile([P, NCH * BS * W], f32, tag="stg", bufs=4, name=f"st{i}")
                for i in range(4)
            ]
            osts = [
                stg_pool.tile([P, NCH * BS * W], f32r, tag="ostg", bufs=4, name=f"ost{i}")
                for i in range(4)
            ]

            nc.vector.memset(ctile[:, :], -65.0)
            nc.vector.memset(sqb[:, :], 11.25)
            for l in range(NL):
                nc.vector.memset(vs[l][:, :], -70.0)
                nc.vector.memset(us[l][:, :], -14.0)
            nc.sync.dma_start(
                bands.rearrange("p (l mc dw two m) -> p l mc dw two m", l=NL, mc=2, dw=3, two=2),
                bands_r,
            )
            nc.sync.dma_start(diags.rearrange("p (d m) -> p d m", d=3), diags_r)
            nc.sync.dma_start(dstage[:, :], diag6r_r)
            nc.scalar.activation(d6r[:, :], dstage[:, :], Act.Copy)
            nc.sync.dma_start(diag6[:, :], diag6_r)
            # zero the act-frame pad columns (wp=0 and wp=257 of each (c,b) row)
            for t in range(T):
                av = acts[t].rearrange("p (c b wp) -> p c b wp", c=NCH, b=BS)
                nc.vector.memset(av[:, :, :, 0:1], 0.0)
                nc.vector.memset(av[:, :, :, FR - 1 : FR], 0.0)

            bands_v = bands.rearrange(
                "p (l mc dw two m) -> p l mc dw two m", l=NL, mc=2, dw=3, two=2
            )
            dv = diags.rearrange("p (d m) -> p d m", d=3)
            d025 = dv[:, 0, :]
            d0001 = dv[:, 1, :]
            d0995 = dv[:, 2, :]

            def interior(t):
                return acts[t].rearrange("p (c b wp) -> p c b wp", c=NCH, b=BS)[
                    :, :, :, 1 : 1 + W
                ]

            def stage_in(t):
                st = sts[t % 4]
                sv = st.rearrange("p (c b w) -> p c b w", c=NCH, b=BS)
                for b in range(BS):
                    nc.sync.dma_start(sv[:, :, b, :], x_r[t, b])
                nc.scalar.activation(interior(t), sv[:, :, :, :], Act.Copy)

            def stage_out(t):
                ov = osts[t % 4].rearrange("p (c b w) -> p c b w", c=NCH, b=BS)
                for b in range(BS):
                    nc.sync.dma_start(o_r[t, b], ov.bitcast(f32)[:, :, b, :])

            def unit_a(l, t):
                v, u, sp = vs[l], us[l], sps[l]
                actv = acts[t].rearrange("p (c b wp) -> p c b wp", c=NCH, b=BS)
                psv = ps_pool.tile([P, NCH * BS * W], f32, tag="psv", name=f"psv_{l}_{t}")
                psu = ps_pool.tile([P, NCH * BS * W], f32, tag="psu", name=f"psu_{l}_{t}")
                psvs[(l, t)] = psv
                psus[(l, t)] = psu

                # conv into psV: per (mc, dw, b) one fp8 DoubleRow mm fusing
                # the kc=0/kc=1 contributions (pair dim = c of the act frame)
                for mc in range(NCH):
                    for dw in range(3):
                        lhsT = bands_v[:, l, mc, dw]         # [P, 2, 128] fp8
                        for b in range(BS):
                            rhs = actv[:, :, b, dw : dw + W]  # [P, 2, 256] fp8
                            nc.tensor.matmul(
                                psv[:, (mc * BS + b) * W : (mc * BS + b + 1) * W],
                                lhsT,
                                rhs,
                                start=(dw == 0),
                                stop=False,
                                perf_mode=DR,
                                skip_group_check=True,
                            )
                # -0.25*u accumulated on top (per chunk), closing each bank
                for mc in range(NCH):
                    sl = slice(mc * BS * W, (mc + 1) * BS * W)
                    nc.tensor.matmul(
                        psv[:, sl], d025, u[:, sl],
                        start=False, stop=True, skip_group_check=True,
                    )
                # psU = 0.001*v_old + 0.995*u_old (reads state pre-overwrite)
                for mc in range(NCH):
                    sl = slice(mc * BS * W, (mc + 1) * BS * W)
                    nc.tensor.matmul(
                        psu[:, sl], d0001, v[:, sl],
                        start=True, stop=False, skip_group_check=True,
                    )
                    nc.tensor.matmul(
                        psu[:, sl], d0995, u[:, sl],
                        start=False, stop=False, skip_group_check=True,
                    )
                # s = 0.01*(v+112.5)^2
                nc.scalar.activation(sp[:, :], v[:, :], Act.Square, bias=sqb[:, 0:1], scale=0.1)
                # v_pre = (s + -91.5625) + psV
                nc.vector.scalar_tensor_tensor(
                    v[:, :], sp[:, :], -91.5625, psv[:, :], Alu.add, Alu.add
                )
                # z: layers 0-2 -> acts[t] fp8; layer 3 -> f32r output staging
                v4 = v.rearrange("p (c b w) -> p c b w", c=NCH, b=BS)
                if l < NL - 1:
                    nc.vector.tensor_scalar(interior(t), v4, 30.0, None, Alu.is_gt)
                else:
                    ov = osts[t % 4].rearrange("p (c b w) -> p c b w", c=NCH, b=BS)
                    nc.vector.tensor_scalar(ov, v4, 30.0, None, Alu.is_gt)

            def unit_b(l, t):
                v, u = vs[l], us[l]
                psu = psus.pop((l, t))
                psvs.pop((l, t))
                if l < NL - 1:
                    zv = interior(t)
                    zmask = zv.bitcast(i8)
                    zlhs, zdt = diag6, None
                else:
                    zv = osts[t % 4].rearrange("p (c b w) -> p c b w", c=NCH, b=BS)
                    zmask = zv.bitcast(mybir.dt.int32)
                    zlhs = d6r
                # psU += 6*z
                for mc in range(NCH):
                    nc.tensor.matmul(
                        psu[:, mc * BS * W : (mc + 1) * BS * W],
                        zlhs[:, :],
                        zv[:, mc, :, :],
                        start=False,
                        stop=True,
                        skip_group_check=True,
                    )
                # reset: v = z ? -65 : v_pre
                v4 = v.rearrange("p (c b w) -> p c b w", c=NCH, b=BS)
                c4 = ctile.rearrange("p (c b w) -> p c b w", c=NCH, b=BS)
                nc.vector.copy_predicated(v4, zmask, c4)
                # u' = psU (0.995u + 0.001v + 6z), PSUM -> SBUF via ACT
                nc.scalar.activation(u[:, :], psu[:, :], Act.Copy)

            psvs, psus = {}, {}
            for k in range(T + NL - 1):
                if k < T:
                    stage_in(k)
                for l in range(NL):
                    t = k - l
                    if 0 <= t < T:
                        unit_a(l, t)
                for l in range(NL):
                    t = k - l
                    if 0 <= t < T:
                        unit_b(l, t)
                to = k - (NL - 1)
                if 0 <= to < T:
                    stage_out(to)

    if not nc.is_finalized():
        nc.finalize()
    return nc


def make_tables(w):
    """w: [4,1,1,3,3] -> fp8 band blocks + f32 diag matmul operands."""
    import ml_dtypes

    fp8 = ml_dtypes.float8_e4m3
    k = (np.asarray(w)[:, 0, 0] * KSC).astype(np.float32)  # [NL,3,3]
    bands = np.zeros((NL, 2, 3, 2, P, P), np.float32)
    for l in range(NL):
        for dw in range(3):
            D = np.zeros((P, P), np.float32)
            for dh in range(3):
                D += k[l, dh, dw] * np.eye(P, k=-(dh - 1), dtype=np.float32)
            B1 = np.zeros((P, P), np.float32)
            B1[P - 1, 0] = k[l, 0, dw]  # out chunk1 row0 <- chunk0 row127
            B2 = np.zeros((P, P), np.float32)
            B2[0, P - 1] = k[l, 2, dw]  # out chunk0 row127 <- chunk1 row0
            bands[l, 0, dw, 0] = D
            bands[l, 0, dw, 1] = B2
            bands[l, 1, dw, 0] = B1
            bands[l, 1, dw, 1] = D
    eye = np.eye(P, dtype=np.float32)
    diags = np.stack([-0.25 * eye, 0.001 * eye, 0.995 * eye]).astype(ml_dtypes.bfloat16)
    diag6 = (6.0 * eye).astype(fp8)
    diag6r = (6.0 * eye).astype(np.float32)
    return bands.astype(fp8), diags, diag6, diag6r


def kernel(x, weights):
    from concourse.bass_utils import run_bass_kernel_spmd

    key = "nc"
    if key not in _CACHE:
        _CACHE[key] = _build()
    nc = _CACHE[key]

    bands, diags, diag6, diag6r = make_tables(np.asarray(weights))
    xs = np.asarray(x)  # [32,16,1,256,256]
    n_cores = 8
    in_maps = []
    for c in range(n_cores):
        shard = np.ascontiguousarray(xs[:, c * BS : (c + 1) * BS, 0])
        in_maps.append({"x": shard, "bands": bands, "diags": diags, "diag6": diag6, "diag6r": diag6r})
    res = run_bass_kernel_spmd(nc, in_maps, core_ids=list(range(n_cores)))
    out = np.zeros((T, 16, 1, H, W), np.float32)
    for c in range(n_cores):
        out[:, c * BS : (c + 1) * BS, 0] = res.results[c]["out"]
    return out
